# revision 5
# baseline (speedup 1.0000x reference)
"""Trainium2 Bass kernel: batched multi-head cross-attention.

Reference computation (per batch element b):
    q = x @ Wq; k,v = split(context @ Wkv)
    per head: attn = softmax(q k^T / 8); o = attn @ v
    out = concat_heads(o) @ Wo + bo

Sharding: pure data parallel - batch B=8, one batch element per NeuronCore,
no collectives. Fully "transposed" dataflow (host feeds x^T / context^T):

    QT[d,i]  = Wq^T  @ xT          KT[d,j] = Wk^T @ cT
    V[j,d]   = cT^T  @ Wv          (ones column fused -> softmax denominator)
    ST[j,i]  = KT_h^T @ QT_h       (per head, contraction d=64)
    PT       = exp(ST / 8)
    OunT,l   = [V_h | 1]^T @ PT
    OnT      = OunT * (1/l)
    outT     = Wo^T @ OnT + bo

Performance structure (vs the naive phase-sequential version):
  * Head PAIRS are processed together; the two heads' S-matmuls have
    K=64 at partition bases 0/64 -> tile_position (0,0)/(64,0), so the PE
    runs them concurrently in disjoint row-groups of the 128x128 array.
  * One shared PSUM pool rotates between S-tiles and projection tiles, so
    the Q/K/V projections are woven INTO the attention pair rounds - the
    scalar engine (exp, the true bottleneck at ~71us) starts ~15us into
    the kernel instead of ~32us.
  * Software pipelining: round jc emits S(jc+1) before O(jc), so the PE
    never head-of-line blocks on the exp of the current round.
  * Matmuls that reuse the immediately-preceding stationary set
    ldweights=False (the self-load is skipped) - LDWEIGHTS was ~39us of
    PE time in the baseline, ~1 load per matmul.
  * Inputs load on all 4 DMA-capable queues (sync/scalar/vector/gpsimd)
    in consumption order; outputs drain round-robin over the 4 queues.
  * Out-projection accumulates hc=0..2 in bulk first (overlapping the last
    pair's normalize chain), then per-(ec,ic): stop-matmul -> bias add
    (alternating vector/scalar) -> store, so the output DMA streams out.
"""

import numpy as np
import ml_dtypes

B, N, M, D = 8, 1024, 1024, 512
H, DH = 8, 64
KC = 4          # 512 contraction -> 4 chunks of 128
IC = 2          # 1024 free dim -> 2 chunks of 512
JC = 8          # 1024 keys -> 8 chunks of 128
NPAIR = 4       # head pairs
N_CORES = 8

DEDUP_LDW = True      # skip redundant stationary self-loads
DEDUP_LDW_S = True    # ...also for the row-tiled S pairs

_BF16 = ml_dtypes.bfloat16
_CACHE = {}
LAST_RUN = None  # BassKernelResults of the most recent launch (for test.py)


def _build_nc():
    import concourse.bass as bass
    import concourse.mybir as mybir
    import concourse.tile as tile
    from concourse import bacc

    f32 = mybir.dt.float32
    bf16 = mybir.dt.bfloat16
    Exp = mybir.ActivationFunctionType.Exp
    Ident = mybir.ActivationFunctionType.Identity

    nc = bacc.Bacc()

    xt = nc.declare_dram_parameter("xt", [D, N], bf16, isOutput=False)
    ct = nc.declare_dram_parameter("ct", [D, M], bf16, isOutput=False)
    wq = nc.declare_dram_parameter("wq", [D, D], bf16, isOutput=False)
    wk = nc.declare_dram_parameter("wk", [D, D], bf16, isOutput=False)
    wv = nc.declare_dram_parameter("wv", [D, D], bf16, isOutput=False)
    wo = nc.declare_dram_parameter("wo", [D, D], bf16, isOutput=False)
    bo = nc.declare_dram_parameter("bo", [D, 1], f32, isOutput=False)
    outT = nc.declare_dram_parameter("outT", [D, N], f32, isOutput=True)

    def dedup(mm):
        if DEDUP_LDW:
            mm.ins.ldweights = False
        return mm

    with tile.TileContext(nc) as tc:
        with (
            tc.tile_pool(name="singles", bufs=1) as singles,
            tc.tile_pool(name="pt", bufs=4) as ptp,
            tc.tile_pool(name="pout", bufs=3) as poutp,
        ):
            def sb(shape, dt, tag):
                return singles.tile(shape, dt, tag=tag, name=tag)

            wq_sb = [sb([128, D], bf16, f"wq{c}") for c in range(KC)]
            xt_sb = [sb([128, N], bf16, f"xt{c}") for c in range(KC)]
            wk_sb = [sb([128, D], bf16, f"wk{c}") for c in range(KC)]
            ct_sb = [sb([128, M], bf16, f"ct{c}") for c in range(KC)]
            wv_sb = [sb([128, D], bf16, f"wv{c}") for c in range(KC)]
            wo_sb = [sb([128, D], bf16, f"wo{c}") for c in range(KC)]
            bo_sb = sb([128, KC, 1], f32, "bo")

            qt_sb = [sb([128, N], bf16, f"qt{c}") for c in range(KC)]
            kt_sb = [sb([128, M], bf16, f"kt{c}") for c in range(KC)]
            v_sb = [sb([128, H, DH + 1], bf16, f"v{j}") for j in range(JC)]
            o_sb = [sb([128, N], bf16, f"o{c}") for c in range(KC)]
            on_sb = [sb([128, N], bf16, f"on{c}") for c in range(KC)]
            # Per-stream broadcast buffers for 1/l (partition_broadcast can
            # only replicate partition 0 downward across all 128 partitions).
            r_sb = [sb([128, N], f32, f"r{s}") for s in range(2)]
            lst = [sb([1, N], f32, f"lst{h}") for h in range(H)]
            linv = [sb([1, N], f32, f"linv{h}") for h in range(H)]

            # ---- input DMA: 3 queues (sync/scalar HWDGE + gpsimd SWDGE),
            # in consumption order.  K-proj consumes ct+wk, Q-proj xt+wq,
            # V-proj ct+wv; wo/bo are only needed at the tail.
            for c in range(KC):
                nc.scalar.dma_start(out=ct_sb[c], in_=ct[c * 128:(c + 1) * 128, :])
            for c in range(KC):
                nc.sync.dma_start(out=wk_sb[c], in_=wk[c * 128:(c + 1) * 128, :])
            for c in range(KC):
                nc.gpsimd.dma_start(out=xt_sb[c], in_=xt[c * 128:(c + 1) * 128, :])
            for c in range(KC):
                nc.sync.dma_start(out=wq_sb[c], in_=wq[c * 128:(c + 1) * 128, :])
            for c in range(KC):
                nc.scalar.dma_start(out=wv_sb[c], in_=wv[c * 128:(c + 1) * 128, :])
            for c in range(KC):
                nc.gpsimd.dma_start(out=wo_sb[c], in_=wo[c * 128:(c + 1) * 128, :])
            nc.sync.dma_start(
                out=bo_sb, in_=bo[:, :].rearrange("(c p) o -> p c o", p=128)
            )

            with (
                tc.tile_pool(name="big", bufs=2, space="PSUM") as bigp,
                tc.tile_pool(name="pso", bufs=1, space="PSUM") as psop,
            ):
                def qk_proj(dst, w_sb, src_sb, dc):
                    t = bigp.tile([128, N], f32, tag="big", name=f"pj{dc}")
                    for kc in range(KC):
                        nc.tensor.matmul(
                            t[:, 0:512],
                            lhsT=w_sb[kc][:, dc * 128:(dc + 1) * 128],
                            rhs=src_sb[kc][:, 0:512],
                            start=(kc == 0), stop=(kc == KC - 1),
                        )
                        dedup(nc.tensor.matmul(
                            t[:, 512:1024],
                            lhsT=w_sb[kc][:, dc * 128:(dc + 1) * 128],
                            rhs=src_sb[kc][:, 512:1024],
                            start=(kc == 0), stop=(kc == KC - 1),
                        ))
                    nc.vector.tensor_copy(dst[dc], t)

                def v_proj_pair(j0):
                    t = bigp.tile([128, N], f32, tag="big", name=f"v{j0}")
                    for half in range(2):
                        jc = j0 + half
                        for kc in range(KC):
                            nc.tensor.matmul(
                                t[:, half * 512:(half + 1) * 512],
                                lhsT=ct_sb[kc][:, jc * 128:(jc + 1) * 128],
                                rhs=wv_sb[kc],
                                start=(kc == 0), stop=(kc == KC - 1),
                            )
                    for half in range(2):
                        jc = j0 + half
                        nc.vector.memset(v_sb[jc][:, :, DH:DH + 1], 1.0)
                        nc.vector.tensor_copy(
                            v_sb[jc][:, :, 0:DH],
                            t[:, half * 512:(half + 1) * 512].rearrange(
                                "p (h d) -> p h d", h=H
                            ),
                        )

                def s_pair(p, jc):
                    # Two K=64 streams at row bases 0/64 -> the PE runs them
                    # concurrently in disjoint row-groups; interleave A/B so
                    # consecutive matmuls never share a row-group.
                    tA = bigp.tile([128, N], f32, tag="big", name=f"sA{p}_{jc}")
                    tB = bigp.tile([128, N], f32, tag="big", name=f"sB{p}_{jc}")
                    for ic in range(IC):
                        for pb, t in ((0, tA), (64, tB)):
                            mm = nc.tensor.matmul(
                                t[:, ic * 512:(ic + 1) * 512],
                                lhsT=kt_sb[p][pb:pb + 64, jc * 128:(jc + 1) * 128],
                                rhs=qt_sb[p][pb:pb + 64, ic * 512:(ic + 1) * 512],
                                start=True, stop=True,
                            )
                            # ic=1 reuses ic=0's stationary; the interleaved
                            # B-load touches only the other row half.
                            if ic == 1 and DEDUP_LDW and DEDUP_LDW_S:
                                mm.ins.ldweights = False
                    return tA, tB

                def exp_pair(p, jc, tA, tB):
                    ptA = ptp.tile([128, N], bf16, tag="pt", name=f"ptA{jc}")
                    ptB = ptp.tile([128, N], bf16, tag="pt", name=f"ptB{jc}")
                    nc.scalar.activation(out=ptA, in_=tA, func=Exp, scale=0.125)
                    nc.scalar.activation(out=ptB, in_=tB, func=Exp, scale=0.125)
                    return ptA, ptB

                def o_round(p, jc, pts, psos):
                    for s in range(2):
                        h = 2 * p + s
                        for ic in range(IC):
                            mm = nc.tensor.matmul(
                                psos[s][0:DH + 1, ic * 512:(ic + 1) * 512],
                                lhsT=v_sb[jc][:, h, :],
                                rhs=pts[s][:, ic * 512:(ic + 1) * 512],
                                start=(jc == 0), stop=(jc == JC - 1),
                            )
                            if ic == 1:
                                dedup(mm)

                qk_proj(kt_sb, wk_sb, ct_sb, 0)
                qk_proj(qt_sb, wq_sb, xt_sb, 0)

                for p in range(NPAIR):
                    psos = (
                        psop.tile([128, N], f32, tag="psoA", name=f"psoA{p}"),
                        psop.tile([128, N], f32, tag="psoB", name=f"psoB{p}"),
                    )
                    pts = {0: exp_pair(p, 0, *s_pair(p, 0))}
                    for jc in range(JC):
                        if p == 0 and jc < 4:
                            v_proj_pair(2 * jc)
                        if jc + 1 < JC:
                            pts[jc + 1] = exp_pair(p, jc + 1, *s_pair(p, jc + 1))
                        elif p + 1 < NPAIR:
                            qk_proj(kt_sb, wk_sb, ct_sb, p + 1)
                            qk_proj(qt_sb, wq_sb, xt_sb, p + 1)
                        o_round(p, jc, pts.pop(jc), psos)
                    # normalize: o / l per head (l = ones-column row of pso)
                    for s in range(2):
                        h, pb = 2 * p + s, 64 * s
                        nc.vector.tensor_copy(lst[h], psos[s][DH:DH + 1, :])
                        nc.vector.reciprocal_approx_fast(out=linv[h], in_=lst[h])
                        nc.vector.tensor_copy(o_sb[p][pb:pb + 64, :], psos[s][0:DH, :])
                        nc.gpsimd.partition_broadcast(r_sb[s], linv[h][0:1, :])
                        nc.vector.tensor_mul(
                            on_sb[p][pb:pb + 64, :],
                            o_sb[p][pb:pb + 64, :],
                            r_sb[s][pb:pb + 64, :],
                        )

            # ---- out-projection: bulk hc=0..2 first (overlaps the last
            # pair's normalize chain), then stop+bias+store per tile ----
            with tc.tile_pool(name="pf", bufs=8, space="PSUM") as pfp:
                psf = [
                    pfp.tile([128, 512], f32, tag="pf", name=f"pf{k}")
                    for k in range(8)
                ]
                for hc in range(KC - 1):
                    for ec in range(KC):
                        for ic in range(IC):
                            mm = nc.tensor.matmul(
                                psf[ec * IC + ic],
                                lhsT=wo_sb[hc][:, ec * 128:(ec + 1) * 128],
                                rhs=on_sb[hc][:, ic * 512:(ic + 1) * 512],
                                start=(hc == 0), stop=False,
                            )
                            if ic == 1:
                                dedup(mm)
                queues = (nc.sync, nc.gpsimd, nc.scalar)
                for ec in range(KC):
                    for ic in range(IC):
                        k = ec * IC + ic
                        mm = nc.tensor.matmul(
                            psf[k],
                            lhsT=wo_sb[3][:, ec * 128:(ec + 1) * 128],
                            rhs=on_sb[3][:, ic * 512:(ic + 1) * 512],
                            start=False, stop=True,
                        )
                        if ic == 1:
                            dedup(mm)
                        ot = poutp.tile([128, 512], f32, tag="pout", name="pout")
                        if k % 2 == 0:
                            nc.vector.tensor_scalar_add(ot, psf[k], bo_sb[:, ec, :])
                        else:
                            nc.scalar.activation(
                                out=ot, in_=psf[k], func=Ident,
                                bias=bo_sb[:, ec, :], scale=1.0,
                            )
                        queues[k % 3].dma_start(
                            out=outT[ec * 128:(ec + 1) * 128, ic * 512:(ic + 1) * 512],
                            in_=ot,
                        )
    nc.finalize()
    return nc


def _ensure_ntff_hook():
    """Install antenv.axon_hooks if the image lacks it, registering the
    ctypes NTFF-profile hook against libaxon_pjrt.so. Without this,
    run_bass_kernel_spmd(trace=True)/BASS_TRACE=1 crashes on import."""
    import contextlib
    import ctypes
    import os
    import sys
    import types

    try:
        import antenv.axon_hooks  # noqa: F401
        return
    except ImportError:
        pass
    try:
        import antenv
    except ImportError:
        return

    state = {"hook": None}
    mod = types.ModuleType("antenv.axon_hooks")
    mod.set_axon_ntff_profile_hook = lambda h: state.__setitem__("hook", h)
    mod.get_axon_ntff_profile_hook = lambda: state["hook"]
    sys.modules["antenv.axon_hooks"] = mod
    antenv.axon_hooks = mod

    so_path = "/opt/axon/libaxon_pjrt.so"
    if not os.path.exists(so_path):
        return
    try:
        lib = ctypes.CDLL(so_path)
    except OSError:
        return
    if not hasattr(lib, "axon_start_nrt_profile"):
        return
    lib.axon_start_nrt_profile.argtypes = [
        ctypes.POINTER(ctypes.c_int64), ctypes.c_size_t,
    ]
    lib.axon_start_nrt_profile.restype = ctypes.c_int64
    lib.axon_stop_nrt_profile.argtypes = [ctypes.c_char_p]
    lib.axon_stop_nrt_profile.restype = ctypes.c_int64

    @contextlib.contextmanager
    def _hook(output_dir, device_ids):
        import jax
        jax.devices()  # force PJRT init so the .so's client exists
        if device_ids:
            ids = (ctypes.c_int64 * len(device_ids))(*device_ids)
            rc = lib.axon_start_nrt_profile(ids, len(device_ids))
        else:
            rc = lib.axon_start_nrt_profile(None, 0)
        if rc != 0:
            raise RuntimeError(f"axon_start_nrt_profile rc={rc}")
        try:
            yield
        finally:
            n = lib.axon_stop_nrt_profile(str(output_dir).encode())
            if n <= 0:
                print(f"ntff profile: rc={n} (no profile output)")

    state["hook"] = _hook


def kernel(x, context, Wq, Wkv, Wo, bo):
    global LAST_RUN
    _ensure_ntff_hook()
    from concourse import bass_utils

    if "nc" not in _CACHE:
        _CACHE["nc"] = _build_nc()
    nc = _CACHE["nc"]

    wq = np.ascontiguousarray(Wq, dtype=np.float32).astype(_BF16)
    wk = np.ascontiguousarray(Wkv[:, :D], dtype=np.float32).astype(_BF16)
    wv = np.ascontiguousarray(Wkv[:, D:], dtype=np.float32).astype(_BF16)
    wo = np.ascontiguousarray(Wo, dtype=np.float32).astype(_BF16)
    bo_ = np.ascontiguousarray(np.asarray(bo, dtype=np.float32).reshape(D, 1))

    in_maps = []
    for b in range(B):
        in_maps.append({
            "xt": np.ascontiguousarray(np.asarray(x[b], np.float32).T).astype(_BF16),
            "ct": np.ascontiguousarray(np.asarray(context[b], np.float32).T).astype(_BF16),
            "wq": wq, "wk": wk, "wv": wv, "wo": wo,
            "bo": bo_,
        })

    LAST_RUN = bass_utils.run_bass_kernel_spmd(nc, in_maps, list(range(N_CORES)))
    out = np.empty((B, N, D), dtype=np.float32)
    for b in range(B):
        out[b] = LAST_RUN.results[b]["outT"].T
    return out
